# revision 41
# baseline (speedup 1.0000x reference)
"""Tensor-parallel Llama GQA attention layer (B=1, S=2048, D=2048, H=32, KV=8)
for 8 Trainium2 NeuronCores.

Sharding: one KV group per core (kv head g + its 4 q heads). Each core computes
its heads' attention and a partial out-projection (contraction over its 256
head-dim columns of wo); the host sums the 8 partials (the TP all-reduce) and
transposes back to [1, S, D].

On-core layout is feature-major (transposed): xt=[D,S], QT=[j,S], KT/VT=[hd,S].
Scores are built per (head, s-superblock of 512, t-block of 128) as
ST=[t,s] tiles; softmax is unnormalized exp (scores are O(1) so no max
subtraction is needed) with the denominator obtained by a ones-column appended
to V, and a single normalization divide at the end.

Perf notes (NTFF-profiled, ~247-254us/core on 8 trn2 NeuronCores; PE matmul
floor is ~139us, ScalarE exp floor ~92us):
- inputs land via a handful of coalesced DMAs (rearranged [dt,p]->[p,dt] APs)
  because every dma_start costs ~600ns of Sync-queue issue time;
- heads run in pairs at PE partition bases 0/64 (roped K duplicated at both
  bases) so score matmuls can overlap in disjoint row strips; both heads'
  score tiles share one 2-bank PSUM tile so a single ScalarE exp covers them
  (halves the 352-cycle per-ACTIVATE overhead), written straight to bf16;
- the causal mask is a bf16 0/1 multiply on the 16 diagonal-superblock tiles;
- UT (attn-out + denominator row) is evacuated to SBUF in one copy so the
  PSUM bank frees immediately; reciprocal_approx_fast + gpsimd
  partition_broadcast + one multiply finish the softmax normalization;
- all PSUM->SBUF moves stay on VectorE (DMA cannot touch PSUM);
- small latency-chained SBUF-to-SBUF DMAs (rope swaps etc) go through the
  gpsimd queue so they cannot head-of-line-block bulk input/output DMAs.
"""

import numpy as np
import ml_dtypes

S = 2048
D = 2048
H = 32
KV = 8
HD = 64
R = 4  # heads per kv group
NC = 8  # cores

BF16 = ml_dtypes.bfloat16


def _build_program(debug_dump=False):
    import concourse.mybir as mybir
    import concourse.tile as tile
    from concourse import bacc

    f32 = mybir.dt.float32
    bf16 = mybir.dt.bfloat16

    nc = bacc.Bacc("TRN2", debug=False, num_devices=NC)
    dbg = {}
    if debug_dump:
        dbg["qtr"] = nc.dram_tensor("dbg_qtr", [128, 2, S], mybir.dt.bfloat16, kind="ExternalOutput")
        dbg["kv2"] = nc.dram_tensor("dbg_kv2", [128, S], mybir.dt.bfloat16, kind="ExternalOutput")
        dbg["vext"] = nc.dram_tensor("dbg_vext", [128, S // 128, 66], mybir.dt.bfloat16, kind="ExternalOutput")
        dbg["at"] = nc.dram_tensor("dbg_at", [128, 2, S], mybir.dt.bfloat16, kind="ExternalOutput")
        dbg["xt"] = nc.dram_tensor("dbg_xt", [128, D // 128, S], mybir.dt.bfloat16, kind="ExternalOutput")
        dbg["kst"] = nc.dram_tensor("dbg_kst", [64, S], mybir.dt.float32, kind="ExternalOutput")

    xt = nc.dram_tensor("xt", [D, S], bf16, kind="ExternalInput")
    wq_t = nc.dram_tensor("wq_t", [D, R * HD], bf16, kind="ExternalInput")
    wkv_t = nc.dram_tensor("wkv_t", [D, 2 * HD], bf16, kind="ExternalInput")
    wo_t = nc.dram_tensor("wo_t", [R * HD, D], bf16, kind="ExternalInput")
    cosb = nc.dram_tensor("cosb", [128, S], bf16, kind="ExternalInput")
    sinb = nc.dram_tensor("sinb", [128, S], bf16, kind="ExternalInput")
    cmask = nc.dram_tensor("cmask", [4, 128, 512], bf16, kind="ExternalInput")
    ident64 = nc.dram_tensor("ident64", [64, 64], bf16, kind="ExternalInput")
    out_t = nc.dram_tensor("out_t", [D, S], f32, kind="ExternalOutput")

    DT = D // 128  # 16 d tiles
    TT = S // 128  # 16 t blocks
    SB = S // 512  # 4 s superblocks

    with tile.TileContext(nc) as tc:
        with (
            tc.tile_pool(name="persist", bufs=1) as persist,
            tc.tile_pool(name="qstage", bufs=2) as qstage_p,
            tc.tile_pool(name="rtmp", bufs=1) as rtmp_p,
            tc.tile_pool(name="et", bufs=6) as etp,
            tc.tile_pool(name="norm", bufs=3) as normp,
            tc.tile_pool(name="ostage", bufs=3) as ostage_p,
            tc.tile_pool(name="qkv_ps", bufs=2, space="PSUM") as qkv_ps,
            tc.tile_pool(name="st_ps", bufs=2, space="PSUM") as st_ps,
            tc.tile_pool(name="ut_ps", bufs=2, space="PSUM") as ut_ps,
        ):
            # ---- persistent SBUF tensors + input DMA ----
            # order matters for the pipeline lead-in: small constants + the
            # first few d-tiles of weights/activations go first so the first
            # projection matmuls can start immediately.
            xt_sb = persist.tile([128, DT, S], bf16)
            wq_sb = persist.tile([128, DT, R * HD], bf16)
            wkv_sb = persist.tile([128, DT, 2 * HD], bf16)
            wo_sb = persist.tile([128, 2, D], bf16)
            cos_sb = persist.tile([128, S], bf16)
            sin_sb = persist.tile([128, S], bf16)
            cmask_sb = persist.tile([128, 4, 512], bf16)
            ident_sb = persist.tile([128, 64], bf16)
            wkv_r = wkv_t.ap().rearrange("(dt p) j -> p dt j", p=128)
            wq_r = wq_t.ap().rearrange("(dt p) j -> p dt j", p=128)
            xt_r = xt.ap().rearrange("(dt p) s -> p dt s", p=128)
            nc.sync.dma_start(out=ident_sb[64:128, :], in_=ident64.ap())
            nc.sync.dma_start(out=wkv_sb, in_=wkv_r)
            nc.sync.dma_start(out=xt_sb[:, :, 0:512], in_=xt_r[:, :, 0:512])
            nc.sync.dma_start(out=wq_sb, in_=wq_r)
            nc.sync.dma_start(out=xt_sb[:, :, 512:1024], in_=xt_r[:, :, 512:1024])
            nc.sync.dma_start(out=cos_sb, in_=cosb.ap())
            nc.sync.dma_start(out=sin_sb, in_=sinb.ap())
            for sc in range(2, 4):
                nc.sync.dma_start(
                    out=xt_sb[:, :, sc * 512:(sc + 1) * 512],
                    in_=xt_r[:, :, sc * 512:(sc + 1) * 512])
            for jj in range(4):
                nc.sync.dma_start(out=cmask_sb[:, jj, :], in_=cmask.ap()[jj])
            for jt in range(2):
                nc.sync.dma_start(out=wo_sb[:, jt, :], in_=wo_t.ap()[jt * 128:(jt + 1) * 128, :])

            qtr_sb = persist.tile([128, 2, S], bf16)   # roped Q, head-major
            kv2_sb = persist.tile([128, S], bf16)      # 0:64 roped K, 64:128 VT
            ko_sb = persist.tile([128, S], bf16)       # 64:128 roped K (odd heads)
            vext_sb = persist.tile([128, TT, 66], bf16)  # V blocks [t,hd] + ones col
            at_sb = persist.tile([128, 2, S], bf16)    # normalized attn out (j-major)

            # ---- RoPE on a 512-col chunk ----
            # within each 64-row head block: rows 0:32 even comps, 32:64 odd comps
            # roped = q * C + swap(q) * S  (C=[cos x4], S=[-sin,+sin]x2, swap 32<->0)
            def rope_chunk(src, dst, nrows, c0, c1):
                swp = rtmp_p.tile([128, 512], bf16, tag="swap")
                for b in range(nrows // 64):
                    nc.gpsimd.dma_start(out=swp[b * 64:b * 64 + 32, :], in_=src[b * 64 + 32:b * 64 + 64, c0:c1])
                    nc.gpsimd.dma_start(out=swp[b * 64 + 32:b * 64 + 64, :], in_=src[b * 64:b * 64 + 32, c0:c1])
                t1 = rtmp_p.tile([128, 512], bf16, tag="ropetmp")
                nc.vector.tensor_mul(t1[:nrows], src[:nrows, c0:c1], cos_sb[0:nrows, c0:c1])
                nc.vector.tensor_mul(swp[:nrows], swp[:nrows], sin_sb[0:nrows, c0:c1])
                nc.vector.tensor_add(dst, t1[:nrows], swp[:nrows])

            # ---- KV projection (K/V feed every head) ----
            kstage = qstage_p.tile([64, S], bf16, tag="kstage")
            qstage_tiles = []
            for _jt in range(2):
                qst_t = qstage_p.tile([128, S], bf16, tag="qstage")
                qstage_tiles.append(qst_t)
            for si in range(SB):
                c0, c1 = si * 512, (si + 1) * 512
                ps = qkv_ps.tile([128, 512], f32, tag="mm")
                for dt in range(DT):
                    nc.tensor.matmul(
                        ps,
                        wkv_sb[:, dt, :],
                        xt_sb[:, dt, c0:c1],
                        start=(dt == 0),
                        stop=(dt == DT - 1),
                    )
                nc.vector.tensor_copy(kstage[:, c0:c1], ps[0:64, :])
                nc.vector.tensor_copy(kv2_sb[64:128, c0:c1], ps[64:128, :])
                rope_chunk(kstage, kv2_sb[0:64, c0:c1], 64, c0, c1)
                # roped K copy at base partition 64 (odd heads), V transposes
                nc.gpsimd.dma_start(out=ko_sb[64:128, c0:c1], in_=kv2_sb[0:64, c0:c1])
                for tt in range(4 * si, 4 * si + 4):
                    vps = st_ps.tile([128, 64], bf16, tag="st")
                    nc.tensor.transpose(vps, kv2_sb[64:128, tt * 128:(tt + 1) * 128], ident_sb[64:128, :])
                    nc.vector.tensor_copy(vext_sb[:, tt, 0:64], vps)
                    nc.vector.memset(vext_sb[:, tt, 64:65], 1.0)

            # ---- Q projection (+ rope per chunk) ----
            for jt in range(2):
                qst = qstage_tiles[jt]
                for si in range(SB):
                    c0, c1 = si * 512, (si + 1) * 512
                    ps = qkv_ps.tile([128, 512], f32, tag="mm")
                    for dt in range(DT):
                        nc.tensor.matmul(
                            ps,
                            wq_sb[:, dt, jt * 128:(jt + 1) * 128],
                            xt_sb[:, dt, c0:c1],
                            start=(dt == 0),
                            stop=(dt == DT - 1),
                        )
                    nc.vector.tensor_copy(qst[:, c0:c1], ps)
                    rope_chunk(qst, qtr_sb[:, jt, c0:c1], 128, c0, c1)

            # ---- attention + partial out-projection, per s superblock ----
            # heads in pairs (2jt, 2jt+1): even head at partition base 0, odd
            # at base 64 -> their MM1s target disjoint PE row strips and run
            # concurrently.
            def emit_outproj(so):
                for dt in range(DT):
                    po = qkv_ps.tile([128, 512], f32, tag="mm")
                    for jt in range(2):
                        nc.tensor.matmul(
                            po,
                            wo_sb[:, jt, dt * 128:(dt + 1) * 128],
                            at_sb[:, jt, so * 512:(so + 1) * 512],
                            start=(jt == 0),
                            stop=(jt == 1),
                        )
                    ost = ostage_p.tile([128, 512], f32)
                    if so == SB - 1 and dt % 2 == 0:
                        nc.scalar.activation(ost, po, mybir.ActivationFunctionType.Copy)
                    else:
                        nc.vector.tensor_copy(ost, po)
                    nc.sync.dma_start(
                        out=out_t.ap()[dt * 128:(dt + 1) * 128, so * 512:(so + 1) * 512],
                        in_=ost,
                    )

            for si in range(SB):
                nblk = 4 * (si + 1)
                c0, c1 = si * 512, (si + 1) * 512
                for jt in range(2):  # head pair (2jt, 2jt+1)
                    qh0 = qtr_sb[0:64, jt, c0:c1]
                    qh1 = qtr_sb[64:128, jt, c0:c1]
                    ut0 = ut_ps.tile([65, 512], f32, tag="ut")
                    ut1 = ut_ps.tile([65, 512], f32, tag="ut")
                    for j in range(nblk):
                        st2 = st_ps.tile([128, 2, 512], f32, tag="st")
                        nc.tensor.matmul(
                            st2[:, 0, :],
                            kv2_sb[0:64, j * 128:(j + 1) * 128],
                            qh0,
                            start=True, stop=True,
                        )
                        nc.tensor.matmul(
                            st2[:, 1, :],
                            ko_sb[64:128, j * 128:(j + 1) * 128],
                            qh1,
                            start=True, stop=True,
                        )
                        et2 = etp.tile([128, 2, 512], bf16, tag="et")
                        nc.scalar.activation(et2, st2, mybir.ActivationFunctionType.Exp)
                        jj = j - 4 * si
                        if jj >= 0:
                            nc.vector.tensor_mul(et2[:, 0, :], et2[:, 0, :], cmask_sb[:, jj, :])
                            nc.vector.tensor_mul(et2[:, 1, :], et2[:, 1, :], cmask_sb[:, jj, :])
                        nc.tensor.matmul(
                            ut0, vext_sb[:, j, 0:65], et2[:, 0, :],
                            start=(j == 0), stop=(j == nblk - 1),
                        )
                        nc.tensor.matmul(
                            ut1, vext_sb[:, j, 0:65], et2[:, 1, :],
                            start=(j == 0), stop=(j == nblk - 1),
                        )
                    # normalize: at = ut[0:64] / ut[64]. First evacuate the
                    # whole ut to SBUF (frees the PSUM slot immediately); the
                    # rest of the chain runs off the critical path.
                    # evacuate ut whole (frees the PSUM slot immediately),
                    # then normalize off the critical path
                    for half, ut in ((0, ut0), (1, ut1)):
                        utsb = normp.tile([65, 512], f32, tag="utsb")
                        nc.vector.tensor_copy(utsb, ut)
                        den0 = normp.tile([1, 512], f32, tag="den0")
                        nc.gpsimd.dma_start(out=den0, in_=utsb[64:65, :])
                        rc = normp.tile([1, 512], f32, tag="recip")
                        nc.vector.reciprocal_approx_fast(rc, den0)
                        bc = normp.tile([64, 512], f32, tag="bcast")
                        nc.gpsimd.partition_broadcast(bc, rc)
                        if half == 0:
                            nc.vector.tensor_mul(
                                at_sb[0:64, jt, c0:c1], utsb[0:64, :], bc)
                        else:
                            tmp64 = normp.tile([64, 512], bf16, tag="tmp64")
                            nc.vector.tensor_mul(tmp64, utsb[0:64, :], bc)
                            nc.gpsimd.dma_start(
                                out=at_sb[64:128, jt, c0:c1], in_=tmp64)

                # partial out-projection for this s superblock
                emit_outproj(si)

            if debug_dump:
                nc.sync.dma_start(out=dbg["qtr"].ap(), in_=qtr_sb)
                nc.sync.dma_start(out=dbg["kv2"].ap(), in_=kv2_sb)
                nc.sync.dma_start(out=dbg["vext"].ap(), in_=vext_sb)
                nc.sync.dma_start(out=dbg["at"].ap(), in_=at_sb)
                nc.sync.dma_start(out=dbg["xt"].ap(), in_=xt_sb)
                nc.sync.dma_start(out=dbg["kst"].ap(), in_=kstage)

    nc.compile()
    return nc


_SIGMA = np.concatenate([np.arange(0, HD, 2), np.arange(1, HD, 2)])


def _prep_inputs(x, freqs_cis, wq, wk, wv, wo):
    """Host-side shard + layout prep. Returns per-core in_maps."""
    x = np.asarray(x, np.float32).reshape(S, D)
    freqs_cis = np.asarray(freqs_cis, np.float32)
    wq = np.asarray(wq, np.float32)
    wk = np.asarray(wk, np.float32)
    wv = np.asarray(wv, np.float32)
    wo = np.asarray(wo, np.float32)

    xt = np.ascontiguousarray(x.T).astype(BF16)

    cosT = np.ascontiguousarray(freqs_cis[:, :, 0].T)  # [32, S]
    sinT = np.ascontiguousarray(freqs_cis[:, :, 1].T)
    cosb = np.ascontiguousarray(np.tile(cosT, (4, 1))).astype(BF16)
    sinb = np.ascontiguousarray(
        np.concatenate([-sinT, sinT, -sinT, sinT], 0)).astype(BF16)

    tloc = np.arange(128)[:, None]
    sloc = np.arange(512)[None, :]
    cmask = np.stack(
        [(128 * jj + tloc <= sloc).astype(np.float32) for jj in range(4)]
    ).astype(BF16)
    ident64 = np.eye(64, dtype=np.float32).astype(BF16)

    scale = 1.0 / np.sqrt(HD)
    in_maps = []
    for g in range(NC):
        wqg = wq[g * R * HD:(g + 1) * R * HD].reshape(R, HD, D)[:, _SIGMA, :].reshape(R * HD, D)
        wq_tg = np.ascontiguousarray(wqg.T).astype(BF16)
        wkg = wk[g * HD:(g + 1) * HD][_SIGMA] * scale
        wvg = wv[g * HD:(g + 1) * HD]
        wkv_tg = np.ascontiguousarray(np.concatenate([wkg, wvg], 0).T).astype(BF16)
        wo_tg = np.ascontiguousarray(wo[:, g * R * HD:(g + 1) * R * HD].T).astype(BF16)
        in_maps.append({
            "xt": xt,
            "wq_t": wq_tg,
            "wkv_t": wkv_tg,
            "wo_t": wo_tg,
            "cosb": cosb,
            "sinb": sinb,
            "cmask": cmask,
            "ident64": ident64,
        })
    return in_maps


_CACHED = {}


def _get_program():
    if "nc" not in _CACHED:
        _CACHED["nc"] = _build_program()
    return _CACHED["nc"]


def kernel(x, freqs_cis, wq, wk, wv, wo, _trace=False):
    from concourse.bass_utils import run_bass_kernel_spmd

    nc = _get_program()
    in_maps = _prep_inputs(x, freqs_cis, wq, wk, wv, wo)
    res = run_bass_kernel_spmd(nc, in_maps, core_ids=list(range(NC)), trace=_trace)
    acc = np.zeros((D, S), np.float64)
    for c in range(NC):
        acc += res.results[c]["out_t"]
    out = np.ascontiguousarray(acc.T, dtype=np.float32).reshape(1, S, D)
    if _trace:
        return out, res
    return out


# revision 42
# speedup vs baseline: 1.1263x; 1.1263x over previous
"""Tensor-parallel Llama GQA attention layer (B=1, S=2048, D=2048, H=32, KV=8)
for 8 Trainium2 NeuronCores.

Sharding: one KV group per core (kv head g + its 4 q heads). Each core computes
its heads' attention and a partial out-projection (contraction over its 256
head-dim columns of wo); the host sums the 8 partials (the TP all-reduce) and
transposes back to [1, S, D].

On-core layout is feature-major (transposed): xt=[D,S], QT=[j,S], KT/VT=[hd,S].
Scores are built per (head, s-superblock of 512, t-block of 128) as
ST=[t,s] tiles; softmax is unnormalized exp (scores are O(1) so no max
subtraction is needed) with the denominator obtained by a ones-column appended
to V, and a single normalization divide at the end.

Perf notes (NTFF-profiled, ~247-254us/core on 8 trn2 NeuronCores; PE matmul
floor is ~139us, ScalarE exp floor ~92us):
- inputs land via a handful of coalesced DMAs (rearranged [dt,p]->[p,dt] APs)
  because every dma_start costs ~600ns of Sync-queue issue time;
- heads run in pairs at PE partition bases 0/64 (roped K duplicated at both
  bases) so score matmuls can overlap in disjoint row strips; both heads'
  score tiles share one 2-bank PSUM tile so a single ScalarE exp covers them
  (halves the 352-cycle per-ACTIVATE overhead), written straight to bf16;
- the causal mask is a bf16 0/1 multiply on the 16 diagonal-superblock tiles;
- UT (attn-out + denominator row) is evacuated to SBUF in one copy so the
  PSUM bank frees immediately; reciprocal_approx_fast + gpsimd
  partition_broadcast + one multiply finish the softmax normalization;
- all PSUM->SBUF moves stay on VectorE (DMA cannot touch PSUM);
- small latency-chained SBUF-to-SBUF DMAs (rope swaps etc) go through the
  gpsimd queue so they cannot head-of-line-block bulk input/output DMAs.
"""

import numpy as np
import ml_dtypes

S = 2048
D = 2048
H = 32
KV = 8
HD = 64
R = 4  # heads per kv group
NC = 8  # cores

BF16 = ml_dtypes.bfloat16


def _build_program(debug_dump=False):
    import concourse.mybir as mybir
    import concourse.tile as tile
    from concourse import bacc

    f32 = mybir.dt.float32
    bf16 = mybir.dt.bfloat16

    nc = bacc.Bacc("TRN2", debug=False, num_devices=NC)
    dbg = {}
    if debug_dump:
        dbg["qtr"] = nc.dram_tensor("dbg_qtr", [128, 2, S], mybir.dt.bfloat16, kind="ExternalOutput")
        dbg["kv2"] = nc.dram_tensor("dbg_kv2", [128, S], mybir.dt.bfloat16, kind="ExternalOutput")
        dbg["vext"] = nc.dram_tensor("dbg_vext", [128, S // 128, 66], mybir.dt.bfloat16, kind="ExternalOutput")
        dbg["at"] = nc.dram_tensor("dbg_at", [128, 2, S], mybir.dt.bfloat16, kind="ExternalOutput")
        dbg["xt"] = nc.dram_tensor("dbg_xt", [128, D // 128, S], mybir.dt.bfloat16, kind="ExternalOutput")
        dbg["kst"] = nc.dram_tensor("dbg_kst", [64, S], mybir.dt.float32, kind="ExternalOutput")

    xt = nc.dram_tensor("xt", [D, S], bf16, kind="ExternalInput")
    wq_t = nc.dram_tensor("wq_t", [D, R * HD], bf16, kind="ExternalInput")
    wkv_t = nc.dram_tensor("wkv_t", [D, 2 * HD], bf16, kind="ExternalInput")
    wo_t = nc.dram_tensor("wo_t", [R * HD, D], bf16, kind="ExternalInput")
    cosb = nc.dram_tensor("cosb", [128, S], bf16, kind="ExternalInput")
    sinb = nc.dram_tensor("sinb", [128, S], bf16, kind="ExternalInput")
    cmask = nc.dram_tensor("cmask", [4, 128, 512], bf16, kind="ExternalInput")
    ident64 = nc.dram_tensor("ident64", [64, 64], bf16, kind="ExternalInput")
    out_t = nc.dram_tensor("out_t", [D, S], f32, kind="ExternalOutput")

    DT = D // 128  # 16 d tiles
    TT = S // 128  # 16 t blocks
    SB = S // 512  # 4 s superblocks

    with tile.TileContext(nc) as tc:
        with (
            tc.tile_pool(name="persist", bufs=1) as persist,
            tc.tile_pool(name="qstage", bufs=2) as qstage_p,
            tc.tile_pool(name="rtmp", bufs=1) as rtmp_p,
            tc.tile_pool(name="et", bufs=6) as etp,
            tc.tile_pool(name="norm", bufs=3) as normp,
            tc.tile_pool(name="ostage", bufs=3) as ostage_p,
            tc.tile_pool(name="qkv_ps", bufs=2, space="PSUM") as qkv_ps,
            tc.tile_pool(name="st_ps", bufs=2, space="PSUM") as st_ps,
            tc.tile_pool(name="ut_ps", bufs=2, space="PSUM") as ut_ps,
        ):
            # ---- persistent SBUF tensors + input DMA ----
            # order matters for the pipeline lead-in: small constants + the
            # first few d-tiles of weights/activations go first so the first
            # projection matmuls can start immediately.
            xt_sb = persist.tile([128, DT, S], bf16)
            wq_sb = persist.tile([128, DT, R * HD], bf16)
            wkv_sb = persist.tile([128, DT, 2 * HD], bf16)
            wo_sb = persist.tile([128, 2, D], bf16)
            cos_sb = persist.tile([128, S], bf16)
            sin_sb = persist.tile([128, S], bf16)
            cmask_sb = persist.tile([128, 4, 512], bf16)
            ident_sb = persist.tile([128, 64], bf16)
            wkv_r = wkv_t.ap().rearrange("(dt p) j -> p dt j", p=128)
            wq_r = wq_t.ap().rearrange("(dt p) j -> p dt j", p=128)
            xt_r = xt.ap().rearrange("(dt p) s -> p dt s", p=128)
            nc.sync.dma_start(out=wkv_sb, in_=wkv_r)
            nc.sync.dma_start(out=xt_sb[:, :, 0:512], in_=xt_r[:, :, 0:512])
            nc.sync.dma_start(out=wq_sb, in_=wq_r)
            nc.sync.dma_start(out=ident_sb[64:128, :], in_=ident64.ap())
            nc.sync.dma_start(out=cos_sb, in_=cosb.ap())
            nc.sync.dma_start(out=sin_sb, in_=sinb.ap())
            for sc in range(1, 4):
                nc.sync.dma_start(
                    out=xt_sb[:, :, sc * 512:(sc + 1) * 512],
                    in_=xt_r[:, :, sc * 512:(sc + 1) * 512])
            for jj in range(4):
                nc.sync.dma_start(out=cmask_sb[:, jj, :], in_=cmask.ap()[jj])
            for jt in range(2):
                nc.sync.dma_start(out=wo_sb[:, jt, :], in_=wo_t.ap()[jt * 128:(jt + 1) * 128, :])

            qtr_sb = persist.tile([128, 2, S], bf16)   # roped Q, head-major
            kv2_sb = persist.tile([128, S], bf16)      # 0:64 roped K, 64:128 VT
            ko_sb = persist.tile([128, S], bf16)       # 64:128 roped K (odd heads)
            vext_sb = persist.tile([128, TT, 66], bf16)  # V blocks [t,hd] + ones col
            at_sb = persist.tile([128, 2, S], bf16)    # normalized attn out (j-major)

            # ---- RoPE on a 512-col chunk ----
            # within each 64-row head block: rows 0:32 even comps, 32:64 odd comps
            # roped = q * C + swap(q) * S  (C=[cos x4], S=[-sin,+sin]x2, swap 32<->0)
            def rope_chunk(src, dst, nrows, c0, c1):
                swp = rtmp_p.tile([128, 512], bf16, tag="swap")
                for b in range(nrows // 64):
                    nc.gpsimd.dma_start(out=swp[b * 64:b * 64 + 32, :], in_=src[b * 64 + 32:b * 64 + 64, c0:c1])
                    nc.gpsimd.dma_start(out=swp[b * 64 + 32:b * 64 + 64, :], in_=src[b * 64:b * 64 + 32, c0:c1])
                t1 = rtmp_p.tile([128, 512], bf16, tag="ropetmp")
                nc.vector.tensor_mul(t1[:nrows], src[:nrows, c0:c1], cos_sb[0:nrows, c0:c1])
                nc.vector.tensor_mul(swp[:nrows], swp[:nrows], sin_sb[0:nrows, c0:c1])
                nc.vector.tensor_add(dst, t1[:nrows], swp[:nrows])

            # ---- KV projection (K/V feed every head) ----
            kstage = qstage_p.tile([64, S], bf16, tag="kstage")
            qstage_tiles = []
            for _jt in range(2):
                qst_t = qstage_p.tile([128, S], bf16, tag="qstage")
                qstage_tiles.append(qst_t)
            for si in range(SB):
                c0, c1 = si * 512, (si + 1) * 512
                ps = qkv_ps.tile([128, 512], f32, tag="mm")
                for dt in range(DT):
                    nc.tensor.matmul(
                        ps,
                        wkv_sb[:, dt, :],
                        xt_sb[:, dt, c0:c1],
                        start=(dt == 0),
                        stop=(dt == DT - 1),
                    )
                nc.vector.tensor_copy(kstage[:, c0:c1], ps[0:64, :])
                nc.vector.tensor_copy(kv2_sb[64:128, c0:c1], ps[64:128, :])
                rope_chunk(kstage, kv2_sb[0:64, c0:c1], 64, c0, c1)
                # roped K copy at base partition 64 (odd heads), V transposes
                nc.gpsimd.dma_start(out=ko_sb[64:128, c0:c1], in_=kv2_sb[0:64, c0:c1])
                for tt in range(4 * si, 4 * si + 4):
                    vps = st_ps.tile([128, 64], bf16, tag="st")
                    nc.tensor.transpose(vps, kv2_sb[64:128, tt * 128:(tt + 1) * 128], ident_sb[64:128, :])
                    nc.vector.tensor_copy(vext_sb[:, tt, 0:64], vps)
                    nc.vector.memset(vext_sb[:, tt, 64:65], 1.0)

            # ---- Q projection (+ rope per chunk) ----
            for jt in range(2):
                qst = qstage_tiles[jt]
                for si in range(SB):
                    c0, c1 = si * 512, (si + 1) * 512
                    ps = qkv_ps.tile([128, 512], f32, tag="mm")
                    for dt in range(DT):
                        nc.tensor.matmul(
                            ps,
                            wq_sb[:, dt, jt * 128:(jt + 1) * 128],
                            xt_sb[:, dt, c0:c1],
                            start=(dt == 0),
                            stop=(dt == DT - 1),
                        )
                    nc.vector.tensor_copy(qst[:, c0:c1], ps)
                    rope_chunk(qst, qtr_sb[:, jt, c0:c1], 128, c0, c1)

            # ---- attention + partial out-projection, per s superblock ----
            # heads in pairs (2jt, 2jt+1): even head at partition base 0, odd
            # at base 64 -> their MM1s target disjoint PE row strips and run
            # concurrently.
            def emit_outproj(so):
                for dt in range(DT):
                    po = qkv_ps.tile([128, 512], f32, tag="mm")
                    for jt in range(2):
                        nc.tensor.matmul(
                            po,
                            wo_sb[:, jt, dt * 128:(dt + 1) * 128],
                            at_sb[:, jt, so * 512:(so + 1) * 512],
                            start=(jt == 0),
                            stop=(jt == 1),
                        )
                    ost = ostage_p.tile([128, 512], f32)
                    if so == SB - 1 and dt % 2 == 0:
                        nc.scalar.activation(ost, po, mybir.ActivationFunctionType.Copy)
                    else:
                        nc.vector.tensor_copy(ost, po)
                    nc.sync.dma_start(
                        out=out_t.ap()[dt * 128:(dt + 1) * 128, so * 512:(so + 1) * 512],
                        in_=ost,
                    )

            for si in range(SB):
                nblk = 4 * (si + 1)
                c0, c1 = si * 512, (si + 1) * 512
                for jt in range(2):  # head pair (2jt, 2jt+1)
                    qh0 = qtr_sb[0:64, jt, c0:c1]
                    qh1 = qtr_sb[64:128, jt, c0:c1]
                    ut0 = ut_ps.tile([65, 512], f32, tag="ut")
                    ut1 = ut_ps.tile([65, 512], f32, tag="ut")
                    for j in range(nblk):
                        st2 = st_ps.tile([128, 2, 512], f32, tag="st")
                        nc.tensor.matmul(
                            st2[:, 0, :],
                            kv2_sb[0:64, j * 128:(j + 1) * 128],
                            qh0,
                            start=True, stop=True,
                        )
                        nc.tensor.matmul(
                            st2[:, 1, :],
                            ko_sb[64:128, j * 128:(j + 1) * 128],
                            qh1,
                            start=True, stop=True,
                        )
                        et2 = etp.tile([128, 2, 512], bf16, tag="et")
                        nc.scalar.activation(et2, st2, mybir.ActivationFunctionType.Exp)
                        jj = j - 4 * si
                        if jj >= 0:
                            nc.vector.tensor_mul(et2[:, 0, :], et2[:, 0, :], cmask_sb[:, jj, :])
                            nc.vector.tensor_mul(et2[:, 1, :], et2[:, 1, :], cmask_sb[:, jj, :])
                        nc.tensor.matmul(
                            ut0, vext_sb[:, j, 0:65], et2[:, 0, :],
                            start=(j == 0), stop=(j == nblk - 1),
                        )
                        nc.tensor.matmul(
                            ut1, vext_sb[:, j, 0:65], et2[:, 1, :],
                            start=(j == 0), stop=(j == nblk - 1),
                        )
                    # normalize: at = ut[0:64] / ut[64]. First evacuate the
                    # whole ut to SBUF (frees the PSUM slot immediately); the
                    # rest of the chain runs off the critical path.
                    # evacuate ut whole (frees the PSUM slot immediately),
                    # then normalize off the critical path
                    for half, ut in ((0, ut0), (1, ut1)):
                        utsb = normp.tile([65, 512], f32, tag="utsb")
                        nc.vector.tensor_copy(utsb, ut)
                        den0 = normp.tile([1, 512], f32, tag="den0")
                        nc.gpsimd.dma_start(out=den0, in_=utsb[64:65, :])
                        rc = normp.tile([1, 512], f32, tag="recip")
                        nc.vector.reciprocal_approx_fast(rc, den0)
                        bc = normp.tile([64, 512], f32, tag="bcast")
                        nc.gpsimd.partition_broadcast(bc, rc)
                        if half == 0:
                            nc.vector.tensor_mul(
                                at_sb[0:64, jt, c0:c1], utsb[0:64, :], bc)
                        else:
                            tmp64 = normp.tile([64, 512], bf16, tag="tmp64")
                            nc.vector.tensor_mul(tmp64, utsb[0:64, :], bc)
                            nc.gpsimd.dma_start(
                                out=at_sb[64:128, jt, c0:c1], in_=tmp64)

                # partial out-projection for this s superblock
                emit_outproj(si)

            if debug_dump:
                nc.sync.dma_start(out=dbg["qtr"].ap(), in_=qtr_sb)
                nc.sync.dma_start(out=dbg["kv2"].ap(), in_=kv2_sb)
                nc.sync.dma_start(out=dbg["vext"].ap(), in_=vext_sb)
                nc.sync.dma_start(out=dbg["at"].ap(), in_=at_sb)
                nc.sync.dma_start(out=dbg["xt"].ap(), in_=xt_sb)
                nc.sync.dma_start(out=dbg["kst"].ap(), in_=kstage)

    nc.compile()
    return nc


_SIGMA = np.concatenate([np.arange(0, HD, 2), np.arange(1, HD, 2)])


def _prep_inputs(x, freqs_cis, wq, wk, wv, wo):
    """Host-side shard + layout prep. Returns per-core in_maps."""
    x = np.asarray(x, np.float32).reshape(S, D)
    freqs_cis = np.asarray(freqs_cis, np.float32)
    wq = np.asarray(wq, np.float32)
    wk = np.asarray(wk, np.float32)
    wv = np.asarray(wv, np.float32)
    wo = np.asarray(wo, np.float32)

    xt = np.ascontiguousarray(x.T).astype(BF16)

    cosT = np.ascontiguousarray(freqs_cis[:, :, 0].T)  # [32, S]
    sinT = np.ascontiguousarray(freqs_cis[:, :, 1].T)
    cosb = np.ascontiguousarray(np.tile(cosT, (4, 1))).astype(BF16)
    sinb = np.ascontiguousarray(
        np.concatenate([-sinT, sinT, -sinT, sinT], 0)).astype(BF16)

    tloc = np.arange(128)[:, None]
    sloc = np.arange(512)[None, :]
    cmask = np.stack(
        [(128 * jj + tloc <= sloc).astype(np.float32) for jj in range(4)]
    ).astype(BF16)
    ident64 = np.eye(64, dtype=np.float32).astype(BF16)

    scale = 1.0 / np.sqrt(HD)
    in_maps = []
    for g in range(NC):
        wqg = wq[g * R * HD:(g + 1) * R * HD].reshape(R, HD, D)[:, _SIGMA, :].reshape(R * HD, D)
        wq_tg = np.ascontiguousarray(wqg.T).astype(BF16)
        wkg = wk[g * HD:(g + 1) * HD][_SIGMA] * scale
        wvg = wv[g * HD:(g + 1) * HD]
        wkv_tg = np.ascontiguousarray(np.concatenate([wkg, wvg], 0).T).astype(BF16)
        wo_tg = np.ascontiguousarray(wo[:, g * R * HD:(g + 1) * R * HD].T).astype(BF16)
        in_maps.append({
            "xt": xt,
            "wq_t": wq_tg,
            "wkv_t": wkv_tg,
            "wo_t": wo_tg,
            "cosb": cosb,
            "sinb": sinb,
            "cmask": cmask,
            "ident64": ident64,
        })
    return in_maps


_CACHED = {}


def _get_program():
    if "nc" not in _CACHED:
        _CACHED["nc"] = _build_program()
    return _CACHED["nc"]


def kernel(x, freqs_cis, wq, wk, wv, wo, _trace=False):
    from concourse.bass_utils import run_bass_kernel_spmd

    nc = _get_program()
    in_maps = _prep_inputs(x, freqs_cis, wq, wk, wv, wo)
    res = run_bass_kernel_spmd(nc, in_maps, core_ids=list(range(NC)), trace=_trace)
    acc = np.zeros((D, S), np.float64)
    for c in range(NC):
        acc += res.results[c]["out_t"]
    out = np.ascontiguousarray(acc.T, dtype=np.float32).reshape(1, S, D)
    if _trace:
        return out, res
    return out


# revision 43
# speedup vs baseline: 1.1463x; 1.0178x over previous
"""Tensor-parallel Llama GQA attention layer (B=1, S=2048, D=2048, H=32, KV=8)
for 8 Trainium2 NeuronCores.

Sharding: one KV group per core (kv head g + its 4 q heads). Each core computes
its heads' attention and a partial out-projection (contraction over its 256
head-dim columns of wo); the host sums the 8 partials (the TP all-reduce) and
transposes back to [1, S, D].

On-core layout is feature-major (transposed): xt=[D,S], QT=[j,S], KT/VT=[hd,S].
Scores are built per (head, s-superblock of 512, t-block of 128) as
ST=[t,s] tiles; softmax is unnormalized exp (scores are O(1) so no max
subtraction is needed) with the denominator obtained by a ones-column appended
to V, and a single normalization divide at the end.

Perf notes (NTFF-profiled, ~247-254us/core on 8 trn2 NeuronCores; PE matmul
floor is ~139us, ScalarE exp floor ~92us):
- inputs land via a handful of coalesced DMAs (rearranged [dt,p]->[p,dt] APs)
  because every dma_start costs ~600ns of Sync-queue issue time;
- heads run in pairs at PE partition bases 0/64 (roped K duplicated at both
  bases) so score matmuls can overlap in disjoint row strips; both heads'
  score tiles share one 2-bank PSUM tile so a single ScalarE exp covers them
  (halves the 352-cycle per-ACTIVATE overhead), written straight to bf16;
- the causal mask is a bf16 0/1 multiply on the 16 diagonal-superblock tiles;
- UT (attn-out + denominator row) is evacuated to SBUF in one copy so the
  PSUM bank frees immediately; reciprocal_approx_fast + gpsimd
  partition_broadcast + one multiply finish the softmax normalization;
- all PSUM->SBUF moves stay on VectorE (DMA cannot touch PSUM);
- small latency-chained SBUF-to-SBUF DMAs (rope swaps etc) go through the
  gpsimd queue so they cannot head-of-line-block bulk input/output DMAs.
"""

import numpy as np
import ml_dtypes

S = 2048
D = 2048
H = 32
KV = 8
HD = 64
R = 4  # heads per kv group
NC = 8  # cores

BF16 = ml_dtypes.bfloat16


def _build_program(debug_dump=False):
    import concourse.mybir as mybir
    import concourse.tile as tile
    from concourse import bacc

    f32 = mybir.dt.float32
    bf16 = mybir.dt.bfloat16

    nc = bacc.Bacc("TRN2", debug=False, num_devices=NC)
    dbg = {}
    if debug_dump:
        dbg["qtr"] = nc.dram_tensor("dbg_qtr", [128, 2, S], mybir.dt.bfloat16, kind="ExternalOutput")
        dbg["kv2"] = nc.dram_tensor("dbg_kv2", [128, S], mybir.dt.bfloat16, kind="ExternalOutput")
        dbg["vext"] = nc.dram_tensor("dbg_vext", [128, S // 128, 66], mybir.dt.bfloat16, kind="ExternalOutput")
        dbg["at"] = nc.dram_tensor("dbg_at", [128, 2, S], mybir.dt.bfloat16, kind="ExternalOutput")
        dbg["xt"] = nc.dram_tensor("dbg_xt", [128, D // 128, S], mybir.dt.bfloat16, kind="ExternalOutput")
        dbg["kst"] = nc.dram_tensor("dbg_kst", [64, S], mybir.dt.float32, kind="ExternalOutput")

    xt = nc.dram_tensor("xt", [D, S], bf16, kind="ExternalInput")
    wq_t = nc.dram_tensor("wq_t", [D, R * HD], bf16, kind="ExternalInput")
    wkv_t = nc.dram_tensor("wkv_t", [D, 2 * HD], bf16, kind="ExternalInput")
    wo_t = nc.dram_tensor("wo_t", [R * HD, D], bf16, kind="ExternalInput")
    cosb = nc.dram_tensor("cosb", [128, S], bf16, kind="ExternalInput")
    sinb = nc.dram_tensor("sinb", [128, S], bf16, kind="ExternalInput")
    cmask = nc.dram_tensor("cmask", [4, 128, 512], bf16, kind="ExternalInput")
    ident64 = nc.dram_tensor("ident64", [64, 64], bf16, kind="ExternalInput")
    out_t = nc.dram_tensor("out_t", [D, S], f32, kind="ExternalOutput")

    DT = D // 128  # 16 d tiles
    TT = S // 128  # 16 t blocks
    SB = S // 512  # 4 s superblocks

    with tile.TileContext(nc) as tc:
        with (
            tc.tile_pool(name="persist", bufs=1) as persist,
            tc.tile_pool(name="qstage", bufs=2) as qstage_p,
            tc.tile_pool(name="rtmp", bufs=1) as rtmp_p,
            tc.tile_pool(name="et", bufs=6) as etp,
            tc.tile_pool(name="norm", bufs=3) as normp,
            tc.tile_pool(name="ostage", bufs=3) as ostage_p,
            tc.tile_pool(name="qkv_ps", bufs=2, space="PSUM") as qkv_ps,
            tc.tile_pool(name="st_ps", bufs=2, space="PSUM") as st_ps,
            tc.tile_pool(name="ut_ps", bufs=2, space="PSUM") as ut_ps,
        ):
            # ---- persistent SBUF tensors + input DMA ----
            # order matters for the pipeline lead-in: small constants + the
            # first few d-tiles of weights/activations go first so the first
            # projection matmuls can start immediately.
            xt_sb = persist.tile([128, DT, S], bf16)
            wq_sb = persist.tile([128, DT, R * HD], bf16)
            wkv_sb = persist.tile([128, DT, 2 * HD], bf16)
            wo_sb = persist.tile([128, 2, D], bf16)
            cos_sb = persist.tile([128, S], bf16)
            sin_sb = persist.tile([128, S], bf16)
            cmask_sb = persist.tile([128, 4, 512], bf16)
            ident_sb = persist.tile([128, 64], bf16)
            wkv_r = wkv_t.ap().rearrange("(dt p) j -> p dt j", p=128)
            wq_r = wq_t.ap().rearrange("(dt p) j -> p dt j", p=128)
            xt_r = xt.ap().rearrange("(dt p) s -> p dt s", p=128)
            nc.sync.dma_start(out=wkv_sb, in_=wkv_r)
            nc.sync.dma_start(out=xt_sb[:, :, 0:512], in_=xt_r[:, :, 0:512])
            nc.sync.dma_start(out=wq_sb, in_=wq_r)
            nc.sync.dma_start(out=ident_sb[64:128, :], in_=ident64.ap())
            nc.sync.dma_start(out=cos_sb, in_=cosb.ap())
            nc.sync.dma_start(out=sin_sb, in_=sinb.ap())
            for sc in range(1, 4):
                nc.sync.dma_start(
                    out=xt_sb[:, :, sc * 512:(sc + 1) * 512],
                    in_=xt_r[:, :, sc * 512:(sc + 1) * 512])
            for jj in range(4):
                nc.sync.dma_start(out=cmask_sb[:, jj, :], in_=cmask.ap()[jj])
            for jt in range(2):
                nc.sync.dma_start(out=wo_sb[:, jt, :], in_=wo_t.ap()[jt * 128:(jt + 1) * 128, :])

            qtr_sb = persist.tile([128, 2, S], bf16)   # roped Q, head-major
            kv2_sb = persist.tile([128, S], bf16)      # 0:64 roped K, 64:128 VT
            ko_sb = persist.tile([128, S], bf16)       # 64:128 roped K (odd heads)
            vext_sb = persist.tile([128, TT, 66], bf16)  # V blocks [t,hd] + ones col
            at_sb = persist.tile([128, 2, S], bf16)    # normalized attn out (j-major)

            # ---- RoPE on a 512-col chunk ----
            # within each 64-row head block: rows 0:32 even comps, 32:64 odd comps
            # roped = q * C + swap(q) * S  (C=[cos x4], S=[-sin,+sin]x2, swap 32<->0)
            def rope_chunk(src, dst, nrows, c0, c1):
                swp = rtmp_p.tile([128, 512], bf16, tag="swap")
                for b in range(nrows // 64):
                    nc.gpsimd.dma_start(out=swp[b * 64:b * 64 + 32, :], in_=src[b * 64 + 32:b * 64 + 64, c0:c1])
                    nc.gpsimd.dma_start(out=swp[b * 64 + 32:b * 64 + 64, :], in_=src[b * 64:b * 64 + 32, c0:c1])
                t1 = rtmp_p.tile([128, 512], bf16, tag="ropetmp")
                nc.vector.tensor_mul(t1[:nrows], src[:nrows, c0:c1], cos_sb[0:nrows, c0:c1])
                nc.vector.tensor_mul(swp[:nrows], swp[:nrows], sin_sb[0:nrows, c0:c1])
                nc.vector.tensor_add(dst, t1[:nrows], swp[:nrows])

            # ---- KV projection (K/V feed every head) ----
            kstage = qstage_p.tile([64, S], bf16, tag="kstage")
            qstage_tiles = []
            for _jt in range(2):
                qst_t = qstage_p.tile([128, S], bf16, tag="qstage")
                qstage_tiles.append(qst_t)
            # per s-chunk: KV proj then both Q projs (all consume the same
            # xt chunk, so PE work covers the chunk's DMA arrival cadence)
            for si in range(SB):
                c0, c1 = si * 512, (si + 1) * 512
                ps = qkv_ps.tile([128, 512], f32, tag="mm")
                for dt in range(DT):
                    nc.tensor.matmul(
                        ps,
                        wkv_sb[:, dt, :],
                        xt_sb[:, dt, c0:c1],
                        start=(dt == 0),
                        stop=(dt == DT - 1),
                    )
                nc.vector.tensor_copy(kstage[:, c0:c1], ps[0:64, :])
                nc.vector.tensor_copy(kv2_sb[64:128, c0:c1], ps[64:128, :])
                rope_chunk(kstage, kv2_sb[0:64, c0:c1], 64, c0, c1)
                # roped K copy at base partition 64 (odd heads), V transposes
                nc.gpsimd.dma_start(out=ko_sb[64:128, c0:c1], in_=kv2_sb[0:64, c0:c1])
                for tt in range(4 * si, 4 * si + 4):
                    vps = st_ps.tile([128, 64], bf16, tag="st")
                    nc.tensor.transpose(vps, kv2_sb[64:128, tt * 128:(tt + 1) * 128], ident_sb[64:128, :])
                    nc.vector.tensor_copy(vext_sb[:, tt, 0:64], vps)
                    nc.vector.memset(vext_sb[:, tt, 64:65], 1.0)
                for jt in range(2):
                    qst = qstage_tiles[jt]
                    ps = qkv_ps.tile([128, 512], f32, tag="mm")
                    for dt in range(DT):
                        nc.tensor.matmul(
                            ps,
                            wq_sb[:, dt, jt * 128:(jt + 1) * 128],
                            xt_sb[:, dt, c0:c1],
                            start=(dt == 0),
                            stop=(dt == DT - 1),
                        )
                    nc.vector.tensor_copy(qst[:, c0:c1], ps)
                    rope_chunk(qst, qtr_sb[:, jt, c0:c1], 128, c0, c1)

            # ---- attention + partial out-projection, per s superblock ----
            # heads in pairs (2jt, 2jt+1): even head at partition base 0, odd
            # at base 64 -> their MM1s target disjoint PE row strips and run
            # concurrently.
            def emit_outproj(so):
                for dt in range(DT):
                    po = qkv_ps.tile([128, 512], f32, tag="mm")
                    for jt in range(2):
                        nc.tensor.matmul(
                            po,
                            wo_sb[:, jt, dt * 128:(dt + 1) * 128],
                            at_sb[:, jt, so * 512:(so + 1) * 512],
                            start=(jt == 0),
                            stop=(jt == 1),
                        )
                    ost = ostage_p.tile([128, 512], f32)
                    if so == SB - 1 and dt % 2 == 0:
                        nc.scalar.activation(ost, po, mybir.ActivationFunctionType.Copy)
                    else:
                        nc.vector.tensor_copy(ost, po)
                    nc.sync.dma_start(
                        out=out_t.ap()[dt * 128:(dt + 1) * 128, so * 512:(so + 1) * 512],
                        in_=ost,
                    )

            for si in range(SB):
                nblk = 4 * (si + 1)
                c0, c1 = si * 512, (si + 1) * 512
                for jt in range(2):  # head pair (2jt, 2jt+1)
                    qh0 = qtr_sb[0:64, jt, c0:c1]
                    qh1 = qtr_sb[64:128, jt, c0:c1]
                    ut0 = ut_ps.tile([65, 512], f32, tag="ut")
                    ut1 = ut_ps.tile([65, 512], f32, tag="ut")
                    for j in range(nblk):
                        st2 = st_ps.tile([128, 2, 512], f32, tag="st")
                        nc.tensor.matmul(
                            st2[:, 0, :],
                            kv2_sb[0:64, j * 128:(j + 1) * 128],
                            qh0,
                            start=True, stop=True,
                        )
                        nc.tensor.matmul(
                            st2[:, 1, :],
                            ko_sb[64:128, j * 128:(j + 1) * 128],
                            qh1,
                            start=True, stop=True,
                        )
                        et2 = etp.tile([128, 2, 512], bf16, tag="et")
                        nc.scalar.activation(et2, st2, mybir.ActivationFunctionType.Exp)
                        jj = j - 4 * si
                        if jj >= 0:
                            nc.vector.tensor_mul(et2[:, 0, :], et2[:, 0, :], cmask_sb[:, jj, :])
                            nc.vector.tensor_mul(et2[:, 1, :], et2[:, 1, :], cmask_sb[:, jj, :])
                        nc.tensor.matmul(
                            ut0, vext_sb[:, j, 0:65], et2[:, 0, :],
                            start=(j == 0), stop=(j == nblk - 1),
                        )
                        nc.tensor.matmul(
                            ut1, vext_sb[:, j, 0:65], et2[:, 1, :],
                            start=(j == 0), stop=(j == nblk - 1),
                        )
                    # normalize: at = ut[0:64] / ut[64]. First evacuate the
                    # whole ut to SBUF (frees the PSUM slot immediately); the
                    # rest of the chain runs off the critical path.
                    # evacuate ut whole (frees the PSUM slot immediately),
                    # then normalize off the critical path
                    for half, ut in ((0, ut0), (1, ut1)):
                        utsb = normp.tile([65, 512], f32, tag="utsb")
                        nc.vector.tensor_copy(utsb, ut)
                        den0 = normp.tile([1, 512], f32, tag="den0")
                        nc.gpsimd.dma_start(out=den0, in_=utsb[64:65, :])
                        rc = normp.tile([1, 512], f32, tag="recip")
                        nc.vector.reciprocal_approx_fast(rc, den0)
                        bc = normp.tile([64, 512], f32, tag="bcast")
                        nc.gpsimd.partition_broadcast(bc, rc)
                        if half == 0:
                            nc.vector.tensor_mul(
                                at_sb[0:64, jt, c0:c1], utsb[0:64, :], bc)
                        else:
                            tmp64 = normp.tile([64, 512], bf16, tag="tmp64")
                            nc.vector.tensor_mul(tmp64, utsb[0:64, :], bc)
                            nc.gpsimd.dma_start(
                                out=at_sb[64:128, jt, c0:c1], in_=tmp64)

                # partial out-projection for this s superblock
                emit_outproj(si)

            if debug_dump:
                nc.sync.dma_start(out=dbg["qtr"].ap(), in_=qtr_sb)
                nc.sync.dma_start(out=dbg["kv2"].ap(), in_=kv2_sb)
                nc.sync.dma_start(out=dbg["vext"].ap(), in_=vext_sb)
                nc.sync.dma_start(out=dbg["at"].ap(), in_=at_sb)
                nc.sync.dma_start(out=dbg["xt"].ap(), in_=xt_sb)
                nc.sync.dma_start(out=dbg["kst"].ap(), in_=kstage)

    nc.compile()
    return nc


_SIGMA = np.concatenate([np.arange(0, HD, 2), np.arange(1, HD, 2)])


def _prep_inputs(x, freqs_cis, wq, wk, wv, wo):
    """Host-side shard + layout prep. Returns per-core in_maps."""
    x = np.asarray(x, np.float32).reshape(S, D)
    freqs_cis = np.asarray(freqs_cis, np.float32)
    wq = np.asarray(wq, np.float32)
    wk = np.asarray(wk, np.float32)
    wv = np.asarray(wv, np.float32)
    wo = np.asarray(wo, np.float32)

    xt = np.ascontiguousarray(x.T).astype(BF16)

    cosT = np.ascontiguousarray(freqs_cis[:, :, 0].T)  # [32, S]
    sinT = np.ascontiguousarray(freqs_cis[:, :, 1].T)
    cosb = np.ascontiguousarray(np.tile(cosT, (4, 1))).astype(BF16)
    sinb = np.ascontiguousarray(
        np.concatenate([-sinT, sinT, -sinT, sinT], 0)).astype(BF16)

    tloc = np.arange(128)[:, None]
    sloc = np.arange(512)[None, :]
    cmask = np.stack(
        [(128 * jj + tloc <= sloc).astype(np.float32) for jj in range(4)]
    ).astype(BF16)
    ident64 = np.eye(64, dtype=np.float32).astype(BF16)

    scale = 1.0 / np.sqrt(HD)
    in_maps = []
    for g in range(NC):
        wqg = wq[g * R * HD:(g + 1) * R * HD].reshape(R, HD, D)[:, _SIGMA, :].reshape(R * HD, D)
        wq_tg = np.ascontiguousarray(wqg.T).astype(BF16)
        wkg = wk[g * HD:(g + 1) * HD][_SIGMA] * scale
        wvg = wv[g * HD:(g + 1) * HD]
        wkv_tg = np.ascontiguousarray(np.concatenate([wkg, wvg], 0).T).astype(BF16)
        wo_tg = np.ascontiguousarray(wo[:, g * R * HD:(g + 1) * R * HD].T).astype(BF16)
        in_maps.append({
            "xt": xt,
            "wq_t": wq_tg,
            "wkv_t": wkv_tg,
            "wo_t": wo_tg,
            "cosb": cosb,
            "sinb": sinb,
            "cmask": cmask,
            "ident64": ident64,
        })
    return in_maps


_CACHED = {}


def _get_program():
    if "nc" not in _CACHED:
        _CACHED["nc"] = _build_program()
    return _CACHED["nc"]


def kernel(x, freqs_cis, wq, wk, wv, wo, _trace=False):
    from concourse.bass_utils import run_bass_kernel_spmd

    nc = _get_program()
    in_maps = _prep_inputs(x, freqs_cis, wq, wk, wv, wo)
    res = run_bass_kernel_spmd(nc, in_maps, core_ids=list(range(NC)), trace=_trace)
    acc = np.zeros((D, S), np.float64)
    for c in range(NC):
        acc += res.results[c]["out_t"]
    out = np.ascontiguousarray(acc.T, dtype=np.float32).reshape(1, S, D)
    if _trace:
        return out, res
    return out
